# revision 122
# baseline (speedup 1.0000x reference)
"""AttentionBlock (GroupNorm -> 1x1 qkv conv -> spatial attention with
softmax over the last width axis -> 1x1 proj conv -> residual) on 8
Trainium2 NeuronCores, data-parallel over the batch.

Self-contained: hardcodes shapes B,C,H,W = 32,512,32,32 and the
8-core batch sharding. Host-side preprocessing folds the C**-0.25
attention scale into the q/k weight rows, transposes the 1x1-conv
weights, and folds the v bias into the proj bias (sum_ij softmax_j(S)
== H exactly). On-device, per sample: GroupNorm stats via channel-wise
bn_stats + a tiny PE select-matmul for the 16-channel group combine
(rsqrt = DVE quake-seed + 2 Newton steps, so only the Exp ACT table is
ever loaded); qkv/scores/attn@v/proj as PE matmuls with the softmax
done in "scores transposed" orientation (ij on partitions, v computed
transposed directly): softmax-over-j denominators are a 128-wide
select-matrix PE matmul accumulated in psum, 1/D is a custom DVE
approx op reading psum directly and writing the bf16 broadcast source,
and the i->32-j-partition broadcast is a replicating DMA (issues
rotated over the SP/ACT/Pool queues) with the A-multiply on DVE. The
scores use the host-folded Gram form S = n^T (Wq'^T Wk') n (+ the
j-dependent bias via the exp's per-partition bias slot; hw-dependent
terms are softmax-invariant and dropped), which removes the separate
q/k projections entirely.

Precision (ATTN_MM_MODE env): "f32r" (default) runs the GN/scores/proj
path in float32r (single-pass PE fp32, 1 cycle/row vs 4 for f32; the
small/odd-shaped matmuls are padded or kept f32 to satisfy the
s3d3_mm_fp32r ISA restrictions) and the attention interior (E, v^T,
softmax select) in bf16 -- rel err ~2e-3 vs the 2e-2 gate, ~234us/core
under the TimelineSim cost model (~3.7x over the all-f32 "f32" mode at
~778us). Scheduling: per-sample phases are software-pipelined (next
sample's GN stats/normalize on Pool, its w/u/v^T matmuls and the
previous sample's deferred proj fill this sample's softmax window);
psum is split 2x2-bank pmm pairs (1024-wide ACT psum->sbuf ops) +
3+1-bank pools for the D/u accumulators, and the broadcast tile pool
is 8-deep so the DMA stream runs ahead of the multiplies, which under
the greedy tile scheduler keeps the PE ~82% busy (~192us of 234us;
startup GN chain and softmax tails account for the rest).
"""

import os
from contextlib import ExitStack

import numpy as np

B, C, H, W = 32, 512, 32, 32
HW = H * W            # 1024
G = 32                # groupnorm groups
GS = C // G           # 16 channels per group
NCORES = 8
BS = B // NCORES      # 4 samples per core
EPS = 1e-5
P = 128
CT = C // P           # 4 channel tiles
IJT = HW // P         # 8 key-pixel tiles
NF = 512              # matmul moving free dim
NH = HW // NF         # 2

# "f32"  : accurate fp32 matmuls (4 cycles/row on PE)
# "f32r" : single-pass fp32 matmuls (1 cycle/row, reduced precision)
MM_MODE = os.environ.get("ATTN_MM_MODE", "f32r")

_cache: dict = {}


def _build(mm_mode: str):
    import concourse.bass as bass
    import concourse.tile as tile
    from concourse import bacc, mybir

    dt = mybir.dt
    AF = mybir.ActivationFunctionType
    ALU = mybir.AluOpType
    f32 = dt.float32
    f32r = dt.float32r
    bf16 = dt.bfloat16
    # matmul-operand dtypes: mdt covers the GN/scores/proj path (f32r =
    # single-pass PE fp32, 4x faster than f32); mdt_att covers the
    # attention interior (E, v^T, softmax select/broadcast) where bf16 is
    # ample (values are softmax weights in [0,1] and normalized v), buys
    # 2x SBUF and the DVE 2-byte fast paths, and keeps 1 PE cycle/row.
    if mm_mode == "f32":
        mdt = mdt_att = f32
    elif mm_mode == "f32r":
        mdt, mdt_att = f32r, bf16
    else:  # mix
        mdt, mdt_att = f32, bf16

    nc = bacc.Bacc("TRN2", target_bir_lowering=False, debug=False,
                   dynamic_dma_scratch_size=8192)

    x_d = nc.dram_tensor("x", [BS, C, HW], f32, kind="ExternalInput").ap()
    mt_d = nc.dram_tensor("mt", [C, C], mdt, kind="ExternalInput").ap()
    id_d = nc.dram_tensor("ident", [P, P], mdt, kind="ExternalInput").ap()
    # vu duplicated to 2 columns: fp32r matmuls need an even moving-dim
    # element count (s3d3_mm_fp32r_restrictions)
    vu_d = nc.dram_tensor("vu", [P, CT, 2], mdt, kind="ExternalInput").ap()
    pt_d = nc.dram_tensor("pt", [C, C], mdt, kind="ExternalInput").ap()
    pb_d = nc.dram_tensor("pb", [P, CT], f32, kind="ExternalInput").ap()
    gw_d = nc.dram_tensor("gw", [P, CT], f32, kind="ExternalInput").ap()
    gb_d = nc.dram_tensor("gb", [P, CT], f32, kind="ExternalInput").ap()
    # selg stays plain f32: its [8,2] matmul output violates the fp32r
    # col_grp==0xf restriction; it is tiny so f32 (4 cyc/row) is free.
    # ssum's free dim is padded 32 -> 128 output rows (rows 32..127 all
    # zero) for the same col_grp restriction; only psum rows 0..31 are
    # read back.
    sg_d = nc.dram_tensor("selg", [P, 8], f32, kind="ExternalInput").ap()
    ss_d = nc.dram_tensor("ssum", [P, IJT * P], mdt_att, kind="ExternalInput").ap()
    out_d = nc.dram_tensor("out", [BS, C, HW], f32, kind="ExternalOutput").ap()

    with tile.TileContext(nc) as tc, ExitStack() as ctx:
        singles = ctx.enter_context(tc.tile_pool(name="singles", bufs=1))
        # pmm tiles are 2-bank [P, NH, NF] pairs so the ACT psum->sbuf ops
        # run 1024-wide (halves ACT op count + per-op init overhead)
        pmm = ctx.enter_context(tc.tile_pool(name="pmm", bufs=2, space="PSUM"))
        pdp = ctx.enter_context(tc.tile_pool(name="pdp", bufs=3, space="PSUM"))
        pup = ctx.enter_context(tc.tile_pool(name="pup", bufs=1, space="PSUM"))

        def pbc(base, rep):
            # partition-broadcast source AP: replicate each source partition
            # `rep` times (destination iterates partitions major)
            base = base.opt(keep_dims={0})
            ap = [d for d in base.ap[1:] if d[1] > 1] or [[1, 1]]
            return bass.AP(
                tensor=base.tensor, offset=base.offset,
                ap=[base.ap[0], [0, rep], *ap],
            )

        # startup DMA priority: xt(0) first (gates GN stats + first matmul),
        # then small constants + the qkv weight, then the remaining x tiles
        # and the proj weight (needed much later)
        xtp = ctx.enter_context(tc.tile_pool(name="xtp", bufs=4))
        xts = {}
        xts[0] = xtp.tile([P, CT, HW], f32, tag="xt", name="xt0")
        xv0 = x_d[0].rearrange("(q p) f -> p q f", p=P)
        # first x tile gates everything: spread its 8 half-chunks across
        # the DMA queues so the startup bn_stats (which run per 512-wide
        # half) can begin as each half lands
        dma_engs = (nc.sync, nc.scalar)
        for h2 in range(2 * CT):
            q, sub = divmod(h2, 2)
            dma_engs[h2 % 2].dma_start(
                xts[0][:, q, sub * NF : (sub + 1) * NF],
                xv0[:, q, sub * NF : (sub + 1) * NF],
            )
        vu_sb = singles.tile([P, CT, 2], mdt)
        nc.scalar.dma_start(vu_sb, vu_d)
        selg_sb = singles.tile([P, 8], f32)
        nc.scalar.dma_start(selg_sb, sg_d)
        pb_sb = singles.tile([P, CT], f32)
        nc.gpsimd.dma_start(pb_sb, pb_d)
        gw_sb = singles.tile([P, CT], f32)
        nc.gpsimd.dma_start(gw_sb, gw_d)
        gb_sb = singles.tile([P, CT], f32)
        nc.gpsimd.dma_start(gb_sb, gb_d)
        ss_sb = singles.tile([P, IJT, P], mdt_att)
        nc.gpsimd.dma_start(ss_sb, ss_d.rearrange("p (t g) -> p t g", t=IJT))

        # mt = Wk'^T Wq' (the scores Gram matrix, host-folded) is needed
        # first, split per o-tile for queue order; ident feeds the PE
        # transpose mode that produces n^T
        mt_sb = singles.tile([P, CT, C], mdt)
        mtv = mt_d.rearrange("(k p) o -> p k o", p=P)
        for ot in range(CT):
            nc.sync.dma_start(
                mt_sb[:, :, ot * P : (ot + 1) * P], mtv[:, :, ot * P : (ot + 1) * P]
            )
        ident_sb = singles.tile([P, P], mdt)
        nc.sync.dma_start(ident_sb, id_d)

        small = ctx.enter_context(tc.tile_pool(name="small", bufs=1))
        stp = ctx.enter_context(tc.tile_pool(name="stp", bufs=4))
        epsb = singles.tile([P, 1], f32)
        nc.vector.memset(epsb, EPS)
        # warm the Exp ACT table set while the first DMAs run (the only
        # table-based ACT function this kernel uses)
        actwarm = singles.tile([P, 1], f32)
        nc.scalar.activation(out=actwarm, in_=epsb, func=AF.Exp)
        magic = singles.tile([8, CT, 1], dt.int32)
        nc.vector.memset(magic, 0x5F3759DF)
        # per-(sample,group) stats: [8 group-in-qtile, (s,q), (mean, E[x^2])]
        gst = singles.tile([8, BS * CT, 2], f32)
        scv = singles.tile([P, BS * CT], f32)
        tcv = singles.tile([P, BS * CT], f32)

        def emit_stats(s):
            """Channel bn_stats on xt(s) -> group combine on PE -> per-channel
            GN scale/offset columns scv/tcv[:, s*CT..]."""
            xt = xts[s]
            for q in range(CT):
                stq = stp.tile([P, 2, 6], f32, tag="stq")
                for sub in range(2):
                    nc.vector.bn_stats(
                        out=stq[:, sub, :], in_=xt[:, q, sub * 512 : (sub + 1) * 512]
                    )
                mvq = stp.tile([P, 2], f32, tag="mvq")
                nc.vector.bn_aggr(out=mvq, in_=stq)
                exq = stp.tile([P, 2], f32, tag="exq")
                nc.vector.tensor_copy(out=exq[:, 0:1], in_=mvq[:, 0:1])
                nc.vector.tensor_scalar(
                    exq[:, 1:2], mvq[:, 0:1], mvq[:, 0:1], mvq[:, 1:2],
                    op0=ALU.mult, op1=ALU.add,
                )
                pg = pdp.tile([8, 2], f32, tag="pd")
                nc.tensor.matmul(pg, lhsT=selg_sb, rhs=exq, start=True, stop=True)
                nc.vector.tensor_copy(out=gst[0:8, s * CT + q, :], in_=pg)
            gm = gst[0:8, s * CT : (s + 1) * CT, 0:1]
            gx2 = gst[0:8, s * CT : (s + 1) * CT, 1:2]
            # the group combine + Newton rsqrt run on Pool: DVE must stay
            # clear for the previous sample's softmax chain (D-copy/recip/
            # A-mult), which otherwise queues behind these ops and stalls PE.
            veng = nc.gpsimd
            gv = stp.tile([8, CT, 1], f32, tag="gv")
            veng.tensor_tensor(gv, gm, gm, ALU.mult)
            veng.tensor_tensor(gv, gx2, gv, ALU.subtract)
            veng.tensor_scalar(gv, gv, EPS, None, op0=ALU.add)
            # rstd = rsqrt(v): quake seed + 3 Newton steps (keeps the
            # stats chain off ACT's table-reload path; ~1e-7 rel)
            # (the int bit-trick seed ops stay on DVE: Pool lacks the
            # shift ALU op; they are 2 tiny instructions)
            i32 = dt.int32
            yb = stp.tile([8, CT, 1], f32, tag="yb")
            nc.vector.tensor_scalar(
                yb.bitcast(i32), gv.bitcast(i32), 1, None,
                op0=ALU.arith_shift_right,
            )
            nc.vector.tensor_tensor(
                yb.bitcast(i32), magic, yb.bitcast(i32), ALU.subtract
            )
            hh = stp.tile([8, CT, 1], f32, tag="hh")
            veng.tensor_scalar(hh, gv, 0.5, None, op0=ALU.mult)
            ttn = stp.tile([8, CT, 1], f32, tag="ttn")
            # 2 Newton steps: seed err ~3% -> ~1e-5 rel, far below the
            # bf16 attention interior's own rounding
            for _ in range(2):
                veng.tensor_tensor(ttn, yb, yb, ALU.mult)
                veng.tensor_tensor(ttn, hh, ttn, ALU.mult)
                veng.tensor_scalar(
                    ttn, ttn, -1.0, 1.5, op0=ALU.mult, op1=ALU.add
                )
                veng.tensor_tensor(yb, yb, ttn, ALU.mult)
            gv = yb
            # replicate each group row to its 16 channel partitions; high
            # priority so these tiny DMAs preempt bulk prefetch wire time
            # in the SP queue (they gate normalize -> w)
            rstdb = stp.tile([P, CT], f32, tag="rstdb")
            with tc.high_priority(offset=1 << 20):
                nc.sync.dma_start(
                    rstdb.opt(keep_dims={0}), pbc(gv[0:8, :, 0], 16)
                )
            gmt = stp.tile([8, CT, 1], f32, tag="gmt")
            veng.tensor_copy(out=gmt, in_=gm)
            gmb = stp.tile([P, CT], f32, tag="gmb")
            with tc.high_priority(offset=1 << 20):
                nc.sync.dma_start(
                    gmb.opt(keep_dims={0}), pbc(gmt[0:8, :, 0], 16)
                )
            cs = scv[:, s * CT : (s + 1) * CT]
            veng.tensor_tensor(cs, gw_sb, rstdb, ALU.mult)
            tmpb = stp.tile([P, CT], f32, tag="tmpb")
            veng.tensor_tensor(tmpb, gmb, cs, ALU.mult)
            veng.tensor_tensor(
                tcv[:, s * CT : (s + 1) * CT], gb_sb, tmpb, ALU.subtract
            )

        emit_stats(0)

        ptmp = ctx.enter_context(tc.tile_pool(name="ptmp", bufs=2))
        bigs = ctx.enter_context(tc.tile_pool(name="bigs", bufs=1))
        rbp = ctx.enter_context(tc.tile_pool(name="rbp", bufs=8))

        # ---- per-sample attention ----
        nts = {}

        def emit_normalize(s):
            # on Pool (same reason as the stats chain above), EXCEPT sample 0
            # where it sits on the startup critical path and DVE is idle
            nt = bigs.tile([P, CT, HW], mdt, tag="nt", bufs=3, name=f"nt{s}")
            nts[s] = nt
            if s == 0:
                # startup critical path: 512-wide halves alternating
                # DVE/Pool so the last chunk lands in ~half the time
                for h2 in range(2 * CT):
                    q, sub = divmod(h2, 2)
                    eng = nc.vector if h2 % 2 == 0 else nc.gpsimd
                    sl = slice(sub * NF, (sub + 1) * NF)
                    eng.tensor_scalar(
                        nt[:, q, sl],
                        xts[s][:, q, sl],
                        scv[:, s * CT + q : s * CT + q + 1],
                        tcv[:, s * CT + q : s * CT + q + 1],
                        op0=ALU.mult,
                        op1=ALU.add,
                    )
                return
            for q in range(CT):
                nc.gpsimd.tensor_scalar(
                    nt[:, q],
                    xts[s][:, q],
                    scv[:, s * CT + q : s * CT + q + 1],
                    tcv[:, s * CT + q : s * CT + q + 1],
                    op0=ALU.mult,
                    op1=ALU.add,
                )

        emit_normalize(0)
        pt_sb = singles.tile([P, CT, C], mdt)
        qks, vts, ess, uss = {}, {}, {}, {}

        def w_groups(s):
            # w[c, ij] = (Wq'^T Wk') n  — the only q/k-side matmul needed:
            # scores are the Gram form S = n^T (Wq'^T Wk') n. Returned as
            # per-psum-pair closures so the caller can interleave them with
            # the previous sample's D matmuls (which pace at the exp stream).
            nt = nts[s]
            wsb = bigs.tile([P, CT, HW], mdt, tag="qk", name=f"w{s}")
            qks[s] = wsb

            def w_one(ot):
                ps = pmm.tile([P, NH, NF], f32, tag="mm")
                for n in range(NH):
                    for k in range(CT):
                        nc.tensor.matmul(
                            ps[:, n],
                            lhsT=mt_sb[:, k, ot * P : (ot + 1) * P],
                            rhs=nt[:, k, n * NF : (n + 1) * NF],
                            start=(k == 0),
                            stop=(k == CT - 1),
                        )
                nc.scalar.activation(
                    out=wsb[:, ot], in_=ps, func=AF.Identity
                )

            return [lambda ot=ot: w_one(ot) for ot in range(CT)]

        def emit_u(s):
            # u[ij] = bq'^T k'[:,ij] (the j-dependent bias term), computed
            # directly in ij-partition layout via N=2 matmuls; applied as
            # the per-partition bias of the scores exp. Emitted after the
            # D matmuls: pu shares the pdp psum pool with the D accumulators
            usb = small.tile([P, IJT], f32, tag="u", bufs=2, name=f"u{s}")
            uss[s] = usb
            nt = nts[s]
            for t in range(IJT):
                pu = pup.tile([P, 2], f32, tag="pu")
                for k in range(CT):
                    nc.tensor.matmul(
                        pu,
                        lhsT=nt[:, k, t * P : (t + 1) * P],
                        rhs=vu_sb[:, k],
                        start=(k == 0),
                        stop=(k == CT - 1),
                    )
                nc.scalar.activation(
                    out=usb[:, t : t + 1], in_=pu[:, 0:1], func=AF.Identity
                )

        def vt_groups(s):
            # n^T[ij, c] via the PE's transpose mode (the value path is
            # host-folded: attn output = (proj_w Wv) (n A), so no v
            # projection is needed on device). Same closure contract as
            # w_groups.
            nt = nts[s]
            vtsb = bigs.tile([P, IJT, C], mdt_att, tag="vt", bufs=2, name=f"ntt{s}")
            vts[s] = vtsb

            def vt_one(t2):
                # transpose output dtype must match lhsT (nt) dtype
                ps = pmm.tile([P, 2, NF], mdt, tag="mm")
                for i2 in range(2):
                    t = 2 * t2 + i2
                    for k in range(CT):
                        nc.tensor.transpose(
                            ps[:, i2, k * P : (k + 1) * P],
                            nt[:, k, t * P : (t + 1) * P],
                            ident_sb,
                        )
                # psum->sbuf (+bf16 round) on ACT: it is idle in the softmax
                # window while DVE carries the D/recip/A-mult chain
                nc.scalar.activation(
                    out=vtsb[:, 2 * t2 : 2 * t2 + 2], in_=ps, func=AF.Identity
                )

            return [lambda t2=t2: vt_one(t2) for t2 in range(IJT // 2)]

        def emit_scores_exp(s):
            # scores transposed S^T[ij, hw] = w^T n; E = exp(S^T + u[ij])
            wsb, nt, usb = qks[s], nts[s], uss[s]
            esb = bigs.tile([P, IJT, HW], mdt_att, tag="E", name=f"E{s}")
            ess[s] = esb
            for t in range(IJT):
                ps = pmm.tile([P, NH, NF], f32, tag="mm")
                for n in range(NH):
                    for k in range(CT):
                        nc.tensor.matmul(
                            ps[:, n],
                            lhsT=wsb[:, k, t * P : (t + 1) * P],
                            rhs=nt[:, k, n * NF : (n + 1) * NF],
                            start=(k == 0),
                            stop=(k == CT - 1),
                        )
                nc.scalar.activation(
                    out=esb[:, t], in_=ps, func=AF.Exp, bias=usb[:, t : t + 1]
                )

        def emit_softmax(s, fillers=()):
            # per-(i,hw) denominators D via select-matrix matmuls (sum the
            # 32 j-partitions, accumulating all 8 ij-tiles into one psum),
            # R = 1/D (custom DVE approx: quake-style fast recip straight
            # from psum + one Newton step writing the bf16 broadcast source
            # directly), then A^T = E * broadcast(R): replicate each i-row
            # of R to its 32 j-partitions with a DMA and multiply on DVE.
            # `fillers` are PE work closures (next sample's w/vt groups)
            # interleaved between D matmuls: the D stream paces at the exp
            # stream's ACT cadence, and the filler keeps the PE busy (and
            # its p-state ramped) in those gaps.
            from concourse.dve_ops import RECIPROCAL_APPROX_NR

            esb = ess[s]
            fillers = list(fillers)
            emitted = 0
            rsc = small.tile([G, HW], f32, tag="rsc")
            rrb = small.tile([G, HW], mdt_att, tag="rrb")
            for n in range(NH):
                pd = pdp.tile([P, NF], f32, tag="pd")
                for t in range(IJT):
                    nc.tensor.matmul(
                        pd,
                        lhsT=ss_sb[:, t, :],
                        rhs=esb[:, t, n * NF : (n + 1) * NF],
                        start=(t == 0),
                        stop=(t == IJT - 1),
                    )
                    done = n * IJT + t + 1
                    want = len(fillers) * done // (NH * IJT)
                    while emitted < want:
                        fillers[emitted]()
                        emitted += 1
                nc.vector.reciprocal_approx_fast(
                    out=rsc[:, n * NF : (n + 1) * NF], in_=pd[0:G]
                )
                nc.vector._custom_dve(
                    RECIPROCAL_APPROX_NR,
                    out=rrb[:, n * NF : (n + 1) * NF],
                    in0=pd[0:G],
                    in1=rsc[:, n * NF : (n + 1) * NF],
                    s0=2.0,
                )
            for f in fillers[emitted:]:
                f()
            for t in range(IJT):
                rbt = rbp.tile([P, HW], mdt_att, tag="rb")
                (nc.sync, nc.scalar, nc.gpsimd)[t % 3].dma_start(
                    rbt, pbc(rrb[4 * t : 4 * t + 4, :], 32)
                )
                # all 8 multiplies on DVE: Pool carries the next sample's
                # GN stats + normalize during this window
                nc.vector.tensor_tensor(esb[:, t], esb[:, t], rbt, ALU.mult)

        def emit_h(s):
            # h[c, hw] = sum_ij v^T[ij,c] * A^T[ij,hw]  (h overwrites nt)
            nt, vtsb, esb = nts[s], vts[s], ess[s]
            for ct in range(CT):
                ps = pmm.tile([P, NH, NF], f32, tag="mm")
                for n in range(NH):
                    for t in range(IJT):
                        nc.tensor.matmul(
                            ps[:, n],
                            lhsT=vtsb[:, t, ct * P : (ct + 1) * P],
                            rhs=esb[:, t, n * NF : (n + 1) * NF],
                            start=(t == 0),
                            stop=(t == IJT - 1),
                        )
                nc.scalar.activation(
                    out=nt[:, ct], in_=ps, func=AF.Identity
                )

        store_engs = (nc.sync, nc.scalar, nc.gpsimd)

        def proj_groups(s):
            # proj + bias + residual (accumulated into xt), then store
            nt, xt = nts[s], xts[s]
            ov = out_d[s].rearrange("(q p) f -> p q f", p=P)

            def p_one(ot):
                ps = pmm.tile([P, NH, NF], f32, tag="mm")
                for n in range(NH):
                    for k in range(CT):
                        nc.tensor.matmul(
                            ps[:, n],
                            lhsT=pt_sb[:, k, ot * P : (ot + 1) * P],
                            rhs=nt[:, k, n * NF : (n + 1) * NF],
                            start=(k == 0),
                            stop=(k == CT - 1),
                        )
                tmp = ptmp.tile([P, HW], f32, tag="pt")
                nc.scalar.activation(
                    out=tmp, in_=ps, func=AF.Identity, bias=pb_sb[:, ot : ot + 1]
                )
                nc.vector.tensor_tensor(xt[:, ot], xt[:, ot], tmp, ALU.add)
                nc.sync.dma_start(ov[:, ot], xt[:, ot])

            return [lambda ot=ot: p_one(ot) for ot in range(CT)]

        def emit_proj(s):
            for f in proj_groups(s):
                f()

        def emit_proj_fine(s):
            # final sample: 512-wide residuals and stores so the tail
            # drains with finer overlap
            nt, xt = nts[s], xts[s]
            ov = out_d[s].rearrange("(q p) f -> p q f", p=P)
            for ot in range(CT):
                ps = pmm.tile([P, NH, NF], f32, tag="mm")
                for n in range(NH):
                    for k in range(CT):
                        nc.tensor.matmul(
                            ps[:, n],
                            lhsT=pt_sb[:, k, ot * P : (ot + 1) * P],
                            rhs=nt[:, k, n * NF : (n + 1) * NF],
                            start=(k == 0),
                            stop=(k == CT - 1),
                        )
                tmp = ptmp.tile([P, HW], f32, tag="pt")
                nc.scalar.activation(
                    out=tmp, in_=ps, func=AF.Identity, bias=pb_sb[:, ot : ot + 1]
                )
                for n in range(NH):
                    sl = slice(n * NF, (n + 1) * NF)
                    nc.vector.tensor_tensor(
                        xt[:, ot, sl], xt[:, ot, sl], tmp[:, sl], ALU.add
                    )
                    (nc.sync if n == 0 else nc.scalar).dma_start(
                        ov[:, ot, sl], xt[:, ot, sl]
                    )

        # software pipeline: the next sample's w/vt matmul groups are
        # interleaved between this sample's D matmuls (which pace at the
        # exp stream's ACT cadence), so the PE never waits for the softmax
        # chain (D -> 1/D -> broadcast -> A-mul) to complete
        for f in w_groups(0):
            f()
        emit_u(0)
        for f in vt_groups(0):
            f()
        projected = set()
        for s in range(BS):
            emit_scores_exp(s)
            if s == 0:
                # deferred + chunked so these bulk bytes interleave with (not
                # block) the small latency-critical startup DMAs in the DMA
                # engines' queue; needed only by proj(0) much later
                ptv = pt_d.rearrange("(k p) o -> p k o", p=P)
                for ot in range(CT):
                    nc.scalar.dma_start(
                        pt_sb[:, :, ot * P : (ot + 1) * P],
                        ptv[:, :, ot * P : (ot + 1) * P],
                    )
            if s + 1 < BS:
                if s + 1 not in xts:
                    xts[s + 1] = xtp.tile(
                        [P, CT, HW], f32, tag="xt", name=f"xt{s + 1}"
                    )
                    xvn = x_d[s + 1].rearrange("(q p) f -> p q f", p=P)
                    for q in range(CT):
                        nc.sync.dma_start(xts[s + 1][:, q], xvn[:, q])
                emit_stats(s + 1)
                emit_normalize(s + 1)
            emit_softmax(s)
            if s + 1 < BS:
                for f in w_groups(s + 1):
                    f()
                emit_u(s + 1)
                for f in vt_groups(s + 1):
                    f()
                if s >= 1:
                    emit_proj(s - 1)
            else:
                emit_proj(s - 1)
            emit_h(s)
        emit_proj_fine(BS - 1)

    nc.compile()
    return nc


def _prep_inputs(x, gn_w, gn_b, qkv_w, qkv_b, proj_w, proj_b):
    x = np.asarray(x, dtype=np.float32)
    gn_w = np.asarray(gn_w, dtype=np.float32)
    gn_b = np.asarray(gn_b, dtype=np.float32)
    qkv_w = np.asarray(qkv_w, dtype=np.float32)
    qkv_b = np.asarray(qkv_b, dtype=np.float32)
    proj_w = np.asarray(proj_w, dtype=np.float32)
    proj_b = np.asarray(proj_b, dtype=np.float32)

    s4 = np.float32(float(C) ** -0.25)
    Wq = (qkv_w[:C] * s4).astype(np.float64)
    Wk = (qkv_w[C : 2 * C] * s4).astype(np.float64)
    bq = (qkv_b[:C] * s4).astype(np.float64)
    # Gram fold: S = n^T (Wq^T Wk) n + (Wk^T bq).n_ij (+ softmax-invariant
    # hw-terms, dropped). mt is the scores lhsT, vu the u-bias vector.
    mt = np.ascontiguousarray((Wk.T @ Wq).astype(np.float32))      # [C, C]
    vu = np.ascontiguousarray(np.repeat(
        (Wk.T @ bq).astype(np.float32).reshape(CT, P).T[:, :, None], 2, axis=2
    ))                                                             # [P, CT, 2]
    Wv = qkv_w[2 * C :].astype(np.float64)
    # value-path fold: attn out = (proj_w Wv) (n A); pt is the lhsT of that
    pt = np.ascontiguousarray(
        (proj_w.astype(np.float64) @ Wv).T.astype(np.float32)
    )                                                              # [C, C]
    ident = np.eye(P, dtype=np.float32)
    vb = qkv_b[2 * C :]
    pb = np.ascontiguousarray(
        (proj_b + np.float32(H) * (proj_w @ vb)).reshape(CT, P).T
    )                                                    # [P, CT]
    gw = np.ascontiguousarray(gn_w.reshape(CT, P).T)   # [P, CT]
    gb = np.ascontiguousarray(gn_b.reshape(CT, P).T)
    selg = np.zeros((P, 8), dtype=np.float32)
    selg[np.arange(P), np.arange(P) // 16] = 1.0 / 16.0
    ss = np.zeros((P, IJT, P), dtype=np.float32)
    for t in range(IJT):
        for p in range(P):
            ss[p, t, 4 * t + p // 32] = 1.0
    ss = np.ascontiguousarray(ss.reshape(P, IJT * P))
    if MM_MODE in ("f32r", "mix"):
        # the device-side "ssum" tensor is bf16 in these modes (0/1 values,
        # exact); ship matching bytes
        import ml_dtypes

        ss = ss.astype(ml_dtypes.bfloat16)
    shared = {
        "mt": mt, "vu": vu, "pt": pt, "pb": pb, "ident": ident,
        "gw": gw, "gb": gb, "ssum": ss, "selg": selg,
    }
    in_maps = []
    for c in range(NCORES):
        m = dict(shared)
        m["x"] = np.ascontiguousarray(x[c * BS : (c + 1) * BS].reshape(BS, C, HW))
        in_maps.append(m)
    return in_maps


def run(inputs: dict, trace: bool = False, n_cores: int = NCORES):
    """Build (cached), run on hardware, return (results, BassKernelResults)."""
    from concourse.bass_utils import run_bass_kernel_spmd

    key = MM_MODE
    if key not in _cache:
        _cache[key] = _build(MM_MODE)
    nc = _cache[key]
    in_maps = _prep_inputs(**inputs)[:n_cores]
    res = run_bass_kernel_spmd(nc, in_maps, list(range(n_cores)), trace=trace)
    return res


def kernel(x, gn_w, gn_b, qkv_w, qkv_b, proj_w, proj_b) -> np.ndarray:
    res = run(dict(x=x, gn_w=gn_w, gn_b=gn_b, qkv_w=qkv_w, qkv_b=qkv_b,
                   proj_w=proj_w, proj_b=proj_b))
    out = np.concatenate(
        [res.results[c]["out"].reshape(BS, C, H, W) for c in range(NCORES)], axis=0
    )
    return out

